# revision 7
# baseline (speedup 1.0000x reference)
"""Trainium2 Bass kernel for nn_BitwiseHashing.

Computes out = tanh(mean_l(x) @ W.T + b) for x:[12,8192,1024] f32,
W:[64,1024], b:[64] -> out:[8192,64].

Strategy (data-parallel over 8 NeuronCores, memory-bound):
  - shard x along batch: 1024 batch cols per core.
  - host casts x to fp16 (rel-err budget 2e-2 leaves ~20x margin) and
    pre-transposes the shard to d-major, l-quad-packed layout
    [3(lq), 1024(d), 4(i)*1024(b)] so that
      * HBM traffic halves (24 MiB/core instead of 48),
      * every DMA is [128, 4096] with 8 KiB contiguous per partition,
      * the summed tile feeds the PE matmul directly as lhsT
        (contraction dim d on partitions) - no transposes at all.
  - per d-chunk (8 of 128 partitions): 3 quad tiles stream in, DVE sums
    them and folds 4096->2048->1024 cols, then 8 matmuls (one per
    128-row batch block) accumulate into a single PSUM bank [128,512]
    across all 8 d-chunks (bias pre-loaded via a C=1 ones-matmul).
  - epilogue: one tanh [128,512] PSUM->SBUF on ACT, one 256 KiB output
    DMA in block-major layout; the host undoes the block permutation.
"""

import numpy as np

import concourse.bacc as bacc
import concourse.mybir as mybir
from concourse import tile
from concourse.bass_utils import run_bass_kernel_spmd

L, B, D, K = 12, 8192, 1024, 64
NCORES = 8
BS = B // NCORES      # 1024 batch columns per core
P = 128               # partitions
NDC = D // P          # 8 contraction chunks
NLQ = 3               # l-quads (12 layers = 3 quads of 4)
QW = 4 * BS           # 4096 cols per quad tile
F32 = mybir.dt.float32
F16 = mybir.dt.float16

_nc_cache = None


def _build():
    global _nc_cache
    if _nc_cache is not None:
        return _nc_cache

    nc = bacc.Bacc("TRN2", target_bir_lowering=False, debug=False)
    x = nc.dram_tensor("x", [NLQ, D, QW], F16, kind="ExternalInput")
    wt = nc.dram_tensor("wt", [D, K], F16, kind="ExternalInput")
    bias = nc.dram_tensor("bias", [1, NDC * K], F16, kind="ExternalInput")
    y = nc.dram_tensor("y", [P, NDC * K], F16, kind="ExternalOutput")

    with tile.TileContext(nc) as tc:
        with (
            tc.tile_pool(name="const", bufs=1) as cpool,
            tc.tile_pool(name="xin", bufs=15) as xpool,
            tc.tile_pool(name="xsl", bufs=4) as spool,
            tc.tile_pool(name="out", bufs=1) as opool,
            tc.tile_pool(name="po", bufs=1, space="PSUM") as ppool,
        ):
            # constants go over the SWDGE queue to keep both HWDGE rings
            # free for the x stream from t=0; bias first (the PE's first
            # emitted instruction waits on it)
            bias_sb = cpool.tile([1, NDC * K], F16)
            nc.gpsimd.dma_start(out=bias_sb[:], in_=bias.ap())
            wt_sb = cpool.tile([P, NDC * K], F16)
            for dc in range(NDC):
                nc.gpsimd.dma_start(
                    out=wt_sb[:, dc * K:(dc + 1) * K],
                    in_=wt.ap()[dc * P:(dc + 1) * P, :],
                )
            ones_sb = cpool.tile([1, P], F16)
            nc.gpsimd.memset(ones_sb[:], 1.0)

            po = ppool.tile([P, NDC * K], F32)
            # bias broadcast across partitions: ones[1,128].T @ bias[1,512]
            nc.tensor.matmul(
                po[:], lhsT=ones_sb[:], rhs=bias_sb[:], start=True, stop=False
            )

            xap = x.ap()
            gctr = [0]  # round-robin over the two HWDGE rings

            def ring():
                eng = nc.sync if gctr[0] % 2 == 0 else nc.scalar
                gctr[0] += 1
                return eng

            def issue_loads(dc):
                d0 = dc * P
                quads, slices = [], []
                for q in range(NLQ):
                    if dc == NDC - 1 and q == NLQ - 1:
                        # last quad arrives as 4 separate l-slices so the
                        # post-stream DVE chain is one [128,1024] add
                        for i in range(4):
                            t = spool.tile([P, BS], F16)
                            ring().dma_start(
                                out=t[:],
                                in_=xap[q, d0:d0 + P, i * BS:(i + 1) * BS],
                            )
                            slices.append(t)
                    else:
                        t = xpool.tile([P, QW], F16)
                        ring().dma_start(out=t[:], in_=xap[q, d0:d0 + P, :])
                        quads.append(t)
                return quads, slices

            def fold_quad(t):
                # fold the 4 l-slices of one quad tile into cols 0:BS;
                # starts as soon as this tile's DMA lands (no cross-tile dep)
                nc.vector.tensor_add(
                    out=t[:, 0:2 * BS], in0=t[:, 0:2 * BS],
                    in1=t[:, 2 * BS:4 * BS],
                )
                nc.vector.tensor_add(
                    out=t[:, 0:BS], in0=t[:, 0:BS], in1=t[:, BS:2 * BS]
                )

            def reduce(ts):
                quads, slices = ts
                acc = quads[0]
                for t in quads:
                    fold_quad(t)
                for t in quads[1:]:
                    nc.vector.tensor_add(
                        out=acc[:, 0:BS], in0=acc[:, 0:BS], in1=t[:, 0:BS]
                    )
                for t in slices:
                    nc.vector.tensor_add(
                        out=acc[:, 0:BS], in0=acc[:, 0:BS], in1=t[:]
                    )
                return acc

            def project(dc, s):
                for blk in range(NDC):
                    nc.tensor.matmul(
                        po[:, blk * K:(blk + 1) * K],
                        lhsT=s[:, blk * P:(blk + 1) * P],
                        rhs=wt_sb[:, dc * K:(dc + 1) * K],
                        start=False,
                        stop=(dc == NDC - 1),
                    )

            PREF = 4  # d-chunks prefetched ahead of the reduce
            tiles = {dc: issue_loads(dc) for dc in range(min(PREF, NDC))}
            for dc in range(NDC):
                s = reduce(tiles.pop(dc))
                if dc + PREF < NDC:
                    tiles[dc + PREF] = issue_loads(dc + PREF)
                project(dc, s)

            ysb = opool.tile([P, NDC * K], F16)
            nc.scalar.activation(
                ysb[:], po[:], mybir.ActivationFunctionType.Tanh
            )
            nc.sync.dma_start(out=y.ap()[:], in_=ysb[:])

    nc.compile()
    _nc_cache = nc
    return nc


def _ensure_ntff_hook():
    """Register the axon NTFF profile hook if the image's antenv lacks it."""
    import sys
    import types

    try:
        from antenv.axon_hooks import get_axon_ntff_profile_hook  # noqa: F401
        return
    except ImportError:
        pass
    import antenv

    mod = types.ModuleType("antenv.axon_hooks")
    mod._hook = None

    def set_axon_ntff_profile_hook(h):
        mod._hook = h

    def get_axon_ntff_profile_hook():
        return mod._hook

    mod.set_axon_ntff_profile_hook = set_axon_ntff_profile_hook
    mod.get_axon_ntff_profile_hook = get_axon_ntff_profile_hook
    sys.modules["antenv.axon_hooks"] = mod
    antenv.axon_hooks = mod
    try:
        from trn_agent_boot.trn_boot import _ntff_profile_via_ctypes

        mod._hook = _ntff_profile_via_ctypes("/opt/axon/libaxon_pjrt.so")
    except Exception:
        mod._hook = None


def _run(inputs, trace=False, **kwargs):
    x = np.asarray(inputs["x"], dtype=np.float32)
    W = np.asarray(inputs["W"], dtype=np.float32)
    b = np.asarray(inputs["b"], dtype=np.float32)
    wt = np.ascontiguousarray(W.T * np.float32(1.0 / L)).astype(np.float16)
    bias = np.tile(b.astype(np.float16), NDC).reshape(1, NDC * K)
    in_maps = []
    for c in range(NCORES):
        xs = x[:, c * BS:(c + 1) * BS, :]            # [12, 1024(b), 1024(d)]
        xq = xs.reshape(NLQ, 4, BS, D).transpose(0, 3, 1, 2)
        xq = np.ascontiguousarray(xq, dtype=np.float16).reshape(NLQ, D, QW)
        in_maps.append({"x": xq, "wt": wt, "bias": bias})
    if trace:
        _ensure_ntff_hook()
        import concourse.bass_utils as bu

        bu.upload_artifacts = lambda tmpdir: "local://skipped"
    nc = _build()
    res = run_bass_kernel_spmd(
        nc, in_maps, core_ids=list(range(NCORES)), trace=trace, **kwargs
    )
    ys = []
    for r in res.results:
        yr = r["y"].astype(np.float32)
        yr = yr.reshape(P, NDC, K).transpose(1, 0, 2).reshape(BS, K)
        ys.append(np.ascontiguousarray(yr))
    return np.concatenate(ys, axis=0), res


def kernel(**inputs):
    y, _ = _run(inputs)
    return y


# revision 8
# speedup vs baseline: 1.1087x; 1.1087x over previous
"""Trainium2 Bass kernel for nn_BitwiseHashing.

Computes out = tanh(mean_l(x) @ W.T + b) for x:[12,8192,1024] f32,
W:[64,1024], b:[64] -> out:[8192,64].

Strategy (data-parallel over 8 NeuronCores, memory-bound):
  - shard x along batch: 1024 batch cols per core.
  - host casts x to fp16 (rel-err budget 2e-2 leaves ~25x margin) and
    pre-transposes the shard to d-major, l-quad-packed layout
    [3(lq), 1024(d), 4(i)*1024(b)] so that
      * HBM traffic halves (24 MiB/core instead of 48),
      * every DMA is [128, 4096] with 8 KiB contiguous per partition,
      * tiles feed the PE matmul directly as k-major rhs
        (contraction dim d on partitions) - no transposes at all.
  - per d-chunk (8 of 128 partitions): 3 quad tiles stream in, DVE sums
    them with two in-place [128,4096] adds (its only job - 60% busy,
    no backlog), then the PE folds the 4 l-slices via 8 accumulating
    512-col matmuls (lhsT = wt chunk [128,64]) into PSUM [64(k),
    1024(b)] shared across all chunks.
  - tail: the last d-chunk's first two quads are loaded FIRST in the
    stream (summed + projected early); its third quad arrives last as
    4 separate [128,1024] l-slices, each feeding 2 matmuls directly -
    after the final byte only ~0.3us of matmul remains, then one
    fused bias+tanh ACT op [64,1024] and a 128 KiB fp16 output DMA.
  - host maps y [64,1024] fp16 back to [1024,64] f32 per core (free).
"""

import numpy as np

import concourse.bacc as bacc
import concourse.mybir as mybir
from concourse import tile
from concourse.bass_utils import run_bass_kernel_spmd

L, B, D, K = 12, 8192, 1024, 64
NCORES = 8
BS = B // NCORES      # 1024 batch columns per core
P = 128               # partitions
NDC = D // P          # 8 contraction chunks
NLQ = 3               # l-quads (12 layers = 3 quads of 4)
QW = 4 * BS           # 4096 cols per quad tile
HF = 512              # one-PSUM-bank matmul width
F32 = mybir.dt.float32
F16 = mybir.dt.float16

_nc_cache = None


def _build():
    global _nc_cache
    if _nc_cache is not None:
        return _nc_cache

    nc = bacc.Bacc("TRN2", target_bir_lowering=False, debug=False)
    x = nc.dram_tensor("x", [NLQ, D, QW], F16, kind="ExternalInput")
    wt = nc.dram_tensor("wt", [D, K], F16, kind="ExternalInput")
    bias = nc.dram_tensor("bias", [K, 1], F32, kind="ExternalInput")
    y = nc.dram_tensor("y", [K, BS], F16, kind="ExternalOutput")

    LAST = NDC - 1

    with tile.TileContext(nc) as tc:
        with (
            tc.tile_pool(name="const", bufs=1) as cpool,
            tc.tile_pool(name="xin", bufs=15) as xpool,
            tc.tile_pool(name="xsl", bufs=4) as spool,
            tc.tile_pool(name="out", bufs=1) as opool,
            tc.tile_pool(name="po", bufs=1, space="PSUM") as ppool,
        ):
            # constants go over the SWDGE queue to keep both HWDGE rings
            # free for the x stream from t=0
            bias_sb = cpool.tile([K, 1], F32)
            nc.gpsimd.dma_start(out=bias_sb[:], in_=bias.ap())
            wt_sb = cpool.tile([P, NDC * K], F16)
            for dc in range(NDC):
                nc.gpsimd.dma_start(
                    out=wt_sb[:, dc * K:(dc + 1) * K],
                    in_=wt.ap()[dc * P:(dc + 1) * P, :],
                )

            po = ppool.tile([K, BS], F32)

            xap = x.ap()
            gctr = [0]   # round-robin over the two HWDGE rings
            mmctr = [0, 0]  # matmuls emitted per PSUM half (start/stop)
            MM_PER_HALF = NDC * 4 + 4  # 7 dcs * 4 + last-dc (4 + 4)

            def ring():
                eng = nc.sync if gctr[0] % 2 == 0 else nc.scalar
                gctr[0] += 1
                return eng

            def load_quad(dc, q):
                d0 = dc * P
                t = xpool.tile([P, QW], F16)
                ring().dma_start(out=t[:], in_=xap[q, d0:d0 + P, :])
                return t

            def load_slices(dc, q):
                d0 = dc * P
                ts = []
                for i in range(4):
                    t = spool.tile([P, BS], F16)
                    ring().dma_start(
                        out=t[:], in_=xap[q, d0:d0 + P, i * BS:(i + 1) * BS]
                    )
                    ts.append(t)
                return ts

            def project(dc, src, i, h):
                # po[k, h*512 + (b%512)] += sum_d src[d, i*1024 + h*512 + b]
                c0 = i * BS + h * HF
                nc.tensor.matmul(
                    po[:, h * HF:(h + 1) * HF],
                    lhsT=wt_sb[:, dc * K:(dc + 1) * K],
                    rhs=src[:, c0:c0 + HF],
                    start=(mmctr[h] == 0),
                    stop=(mmctr[h] == MM_PER_HALF - 1),
                )
                mmctr[h] += 1

            def project_quad(dc, src):
                for i in range(4):
                    for h in range(2):
                        project(dc, src, i, h)

            # ---- emission ----
            # stream order: LAST dc's quads 0,1 first, then dc0..dc6,
            # then LAST dc's quad 2 as 4 l-slices (the kernel's tail).
            u7a = load_quad(LAST, 0)
            u7b = load_quad(LAST, 1)
            PREF = 3
            tiles = {dc: [load_quad(dc, q) for q in range(NLQ)]
                     for dc in range(min(PREF, NDC - 1))}

            nc.vector.tensor_add(out=u7a[:], in0=u7a[:], in1=u7b[:])
            project_quad(LAST, u7a)

            slices = None
            for dc in range(NDC - 1):
                t0, t1, t2 = tiles.pop(dc)
                nc.vector.tensor_add(out=t0[:], in0=t0[:], in1=t1[:])
                nc.vector.tensor_add(out=t0[:], in0=t0[:], in1=t2[:])
                if dc + PREF < NDC - 1:
                    tiles[dc + PREF] = [
                        load_quad(dc + PREF, q) for q in range(NLQ)
                    ]
                elif slices is None:
                    slices = load_slices(LAST, 2)
                project_quad(dc, t0)

            for t in slices:
                for h in range(2):
                    project(LAST, t, 0, h)

            ysb = opool.tile([K, BS], F16)
            nc.scalar.activation(
                ysb[:], po[:], mybir.ActivationFunctionType.Tanh,
                bias=bias_sb[:],
            )
            nc.sync.dma_start(out=y.ap()[:], in_=ysb[:])

    nc.compile()
    _nc_cache = nc
    return nc


def _ensure_ntff_hook():
    """Register the axon NTFF profile hook if the image's antenv lacks it."""
    import sys
    import types

    try:
        from antenv.axon_hooks import get_axon_ntff_profile_hook  # noqa: F401
        return
    except ImportError:
        pass
    import antenv

    mod = types.ModuleType("antenv.axon_hooks")
    mod._hook = None

    def set_axon_ntff_profile_hook(h):
        mod._hook = h

    def get_axon_ntff_profile_hook():
        return mod._hook

    mod.set_axon_ntff_profile_hook = set_axon_ntff_profile_hook
    mod.get_axon_ntff_profile_hook = get_axon_ntff_profile_hook
    sys.modules["antenv.axon_hooks"] = mod
    antenv.axon_hooks = mod
    try:
        from trn_agent_boot.trn_boot import _ntff_profile_via_ctypes

        mod._hook = _ntff_profile_via_ctypes("/opt/axon/libaxon_pjrt.so")
    except Exception:
        mod._hook = None


def _run(inputs, trace=False, **kwargs):
    x = np.asarray(inputs["x"], dtype=np.float32)
    W = np.asarray(inputs["W"], dtype=np.float32)
    b = np.asarray(inputs["b"], dtype=np.float32)
    wt = np.ascontiguousarray(W.T * np.float32(1.0 / L)).astype(np.float16)
    bias = np.ascontiguousarray(b.reshape(K, 1)).astype(np.float32)
    in_maps = []
    for c in range(NCORES):
        xs = x[:, c * BS:(c + 1) * BS, :]            # [12, 1024(b), 1024(d)]
        xq = xs.reshape(NLQ, 4, BS, D).transpose(0, 3, 1, 2)
        xq = np.ascontiguousarray(xq, dtype=np.float16).reshape(NLQ, D, QW)
        in_maps.append({"x": xq, "wt": wt, "bias": bias})
    if trace:
        _ensure_ntff_hook()
        import concourse.bass_utils as bu

        bu.upload_artifacts = lambda tmpdir: "local://skipped"
    nc = _build()
    res = run_bass_kernel_spmd(
        nc, in_maps, core_ids=list(range(NCORES)), trace=trace, **kwargs
    )
    ys = []
    for r in res.results:
        yr = r["y"].astype(np.float32).T           # [1024(b), 64(k)]
        ys.append(np.ascontiguousarray(yr))
    return np.concatenate(ys, axis=0), res


def kernel(**inputs):
    y, _ = _run(inputs)
    return y
